# revision 1
# baseline (speedup 1.0000x reference)
"""GQA kernel for trn2: B=2, L=2048, D=2048, Hq=32, Hkv=8, dh=64.

Sharding: 1 KV head (= 4 contiguous Q heads) per core; Wq/Wk/Wv
column-sharded by head, Wo row-sharded; partial outputs summed on host.

Layout trick: x is transposed on the host (xT: [D, B*L]) so every
on-device matmul has its contraction dim on partitions without any
on-device transposes:
  Q^T[dq, l]  = (Wq_tile).T @ xT        (lhsT=Wq, rhs=xT)
  K^T[dh, l]  = (Wk_tile).T @ xT
  V[l, dh]    = (xT_tile).T @ Wv        (lhsT=xT, rhs=Wv)
  S^T[k, q]   = (K^T_tile).T @ Q^T      (lhsT=K^T, rhs=Q^T)   contract dh=64
  E           = exp(S^T / 8)            (ScalarE, PSUM->SBUF)
  U[0:65, q]  = [V|1].T @ E             (lhsT=V_aug, rhs=E)   contract Lk
                row 64 of U = softmax denominator (ones column trick)
  attnT       = U[:64] * bcast(1/U[64]) (DVE recip + K=1 matmul bcast + mul)
  out[l, :]  += (attnT_tile).T @ Wo     (lhsT=attnT, rhs=Wo)
"""

import ml_dtypes
import numpy as np

import concourse.bass as bass
import concourse.bacc as bacc
import concourse.mybir as mybir
from concourse.tile import TileContext, add_dep_helper
from concourse.bass_utils import run_bass_kernel_spmd

B, L, D = 2, 2048, 2048
HQ, HKV, DH = 32, 8, 64
GQ = HQ // HKV            # 4 q heads per core
DQ = GQ * DH              # 256
BL = B * L                # 4096
P = 128
NB = 512                  # free-dim block
KD = D // P               # 16 contraction tiles over D
LT = L // P               # 16 Lk tiles per batch
NBLK = L // NB            # 4 Lq blocks per batch
SCALE = 1.0 / 8.0         # 1/sqrt(dh)

F32 = mybir.dt.float32
BF16 = mybir.dt.bfloat16
AF = mybir.ActivationFunctionType

_CACHED = {}


def _pe_sync(nc, producers, reason):
    # Hoist multi-source waits onto a PE nop: the self-loading f32r matmul
    # (S3_LW) can only carry a single sync wait in walrus codegen.
    if not producers:
        return
    nop = nc.tensor.nop(nofuse=True, hint="sponge")
    for p in producers:
        add_dep_helper(nop.ins, p.ins, reason=reason)


def build_nc():
    nc = bacc.Bacc()
    xT = nc.declare_dram_parameter("xT", [D, BL], BF16, isOutput=False)
    wq = nc.declare_dram_parameter("wq", [D, DQ], BF16, isOutput=False)
    wk = nc.declare_dram_parameter("wk", [D, 2 * DH], BF16, isOutput=False)
    wv = nc.declare_dram_parameter("wv", [D, DH], BF16, isOutput=False)
    wo = nc.declare_dram_parameter("wo", [DQ, D], BF16, isOutput=False)
    out = nc.declare_dram_parameter("out", [BL, D], F32, isOutput=True)

    with TileContext(nc) as tc:
        with (
            tc.tile_pool(name="wpool", bufs=1) as wpool,
            tc.tile_pool(name="xpool", bufs=3) as xpool,
            tc.tile_pool(name="qtpool", bufs=3) as qtpool,
            tc.tile_pool(name="ktpool", bufs=2) as ktpool,
            tc.tile_pool(name="vpool", bufs=34) as vpool,
            tc.tile_pool(name="epool", bufs=20) as epool,
            tc.tile_pool(name="atpool", bufs=2) as atpool,
            tc.tile_pool(name="opool", bufs=3) as opool,
            tc.tile_pool(name="bcpool", bufs=2) as bcpool,
            tc.tile_pool(name="rpool", bufs=4) as rpool,
            tc.tile_pool(name="psA", bufs=2, space="PSUM") as psA,
            tc.tile_pool(name="psS", bufs=4, space="PSUM") as psS,
            tc.tile_pool(name="psU", bufs=2, space="PSUM") as psU,
        ):
            # ---- persistent weights ----
            wdmas = []
            wq_sb = wpool.tile([P, KD, DQ], BF16, tag="wq")
            wdmas.append(nc.sync.dma_start(out=wq_sb, in_=wq.rearrange("(k p) m -> p k m", p=P)))
            wk_sb = wpool.tile([P, KD, 2 * DH], BF16, tag="wk")
            wdmas.append(nc.sync.dma_start(out=wk_sb, in_=wk.rearrange("(k p) m -> p k m", p=P)))
            wv_sb = wpool.tile([P, KD, DH], BF16, tag="wv")
            wdmas.append(nc.sync.dma_start(out=wv_sb, in_=wv.rearrange("(k p) m -> p k m", p=P)))
            wo_sb = [wpool.tile([P, D], BF16, tag=f"wo{t}", name=f"wo_sb{t}") for t in range(2)]
            for t in range(2):
                wdmas.append(nc.sync.dma_start(out=wo_sb[t], in_=wo[t * P : (t + 1) * P, :]))
            ones_sb = wpool.tile([1, DH], BF16, tag="ones")
            nc.vector.memset(ones_sb, 1.0)

            for b in range(B):
                # ---------- phase A: projections for batch b ----------
                qt_sb = [qtpool.tile([P, L], BF16, tag="qt", name=f"qt_sb{t}") for t in range(2)]
                kt_sb = ktpool.tile([P, L], BF16, tag="kt")
                v_sb = [vpool.tile([P, DH + 1], BF16, tag="v", name=f"v_sb{k}") for k in range(LT)]
                acopies = []

                for c in range(NBLK):
                    c0 = b * L + c * NB  # column offset in BL
                    xt_all = xpool.tile([P, KD, NB], BF16, tag="xt")
                    xdma = nc.sync.dma_start(
                        out=xt_all,
                        in_=xT.rearrange("(k p) n -> p k n", p=P)[:, :, c0 : c0 + NB],
                    )

                    # Q^T (two 128-row dq tiles)
                    for t in range(2):
                        q_ps = psA.tile([P, NB], F32, tag="acc")
                        for k in range(KD):
                            nc.tensor.matmul(
                                q_ps,
                                lhsT=wq_sb[:, k, t * P : (t + 1) * P],
                                rhs=xt_all[:, k, :],
                                start=(k == 0),
                                stop=(k == KD - 1),
                            )
                        acopies.append(nc.vector.tensor_copy(
                            qt_sb[t][:, c * NB : (c + 1) * NB], q_ps
                        ))
                    # K^T
                    k_ps = psA.tile([P, NB], F32, tag="acc")
                    for k in range(KD):
                        nc.tensor.matmul(
                            k_ps,
                            lhsT=wk_sb[:, k, :],
                            rhs=xt_all[:, k, :],
                            start=(k == 0),
                            stop=(k == KD - 1),
                        )
                    acopies.append(nc.vector.tensor_copy(kt_sb[:, c * NB : (c + 1) * NB], k_ps))
                    # V (natural, Lk-major) + ones column
                    for j in range(NB // P):
                        lk = c * (NB // P) + j
                        v_ps = psA.tile([P, DH], F32, tag="acc")
                        for k in range(KD):
                            nc.tensor.matmul(
                                v_ps,
                                lhsT=xt_all[:, k, j * P : (j + 1) * P],
                                rhs=wv_sb[:, k, :],
                                start=(k == 0),
                                stop=(k == KD - 1),
                            )
                        acopies.append(nc.vector.tensor_copy(v_sb[lk][:, :DH], v_ps))
                        acopies.append(nc.vector.memset(v_sb[lk][:, DH : DH + 1], 1.0))

                # ---------- phases B+C per Lq block ----------
                for c in range(NBLK):
                    at_sb = [atpool.tile([P, NB], BF16, tag="at", name=f"at_sb{t}") for t in range(2)]
                    at_producers = []
                    for g in range(GQ):
                        qg = qt_sb[g // 2][
                            (g % 2) * DH : (g % 2) * DH + DH, c * NB : (c + 1) * NB
                        ]
                        # S^T tiles + exp; interleave PV to keep PE/ACT in step
                        e_sb = []
                        sT_live = []
                        u_ps = psU.tile([P, NB], F32, tag="u")

                        h0 = (g % 2) * DH

                        def qk_step(k):
                            sT = psS.tile([P, NB], F32, tag="sT")
                            nc.tensor.matmul(
                                sT,
                                lhsT=kt_sb[h0 : h0 + DH, k * P : (k + 1) * P],
                                rhs=qg,
                                start=True,
                                stop=True,
                            )
                            e = epool.tile([P, NB], BF16, tag="e")
                            nc.scalar.activation(e, sT, AF.Exp, scale=SCALE)
                            e_sb.append(e)

                        def pv_step(k):
                            nc.tensor.matmul(
                                u_ps[: DH + 1, :],
                                lhsT=v_sb[c * 0 + k][:, :],
                                rhs=e_sb[k],
                                start=(k == 0),
                                stop=(k == LT - 1),
                            )

                        for k in range(4):
                            qk_step(k)
                        for k in range(4, LT):
                            qk_step(k)
                            pv_step(k - 4)
                        for k in range(LT - 4, LT):
                            pv_step(k)

                        # normalize: attnT = U[:64] * bcast(1 / U[64])
                        recip = rpool.tile([1, NB], BF16, tag="r")
                        with nc.allow_low_precision(reason="f32r is fp32-width"):
                            nc.vector.reciprocal(recip, u_ps[DH : DH + 1, :])
                        bc_ps = psS.tile([DH, NB], F32, tag="sT")
                        nc.tensor.matmul(
                            bc_ps, lhsT=ones_sb, rhs=recip, start=True, stop=True
                        )
                        bc_sb = bcpool.tile([DH, NB], F32, tag="bc")
                        nc.vector.tensor_copy(bc_sb, bc_ps)
                        if g % 2 == 0:
                            at_producers.append(nc.vector.tensor_mul(
                                at_sb[g // 2][:DH, :], u_ps[:DH, :], bc_sb
                            ))
                        else:
                            at_tmp = rpool.tile([DH, NB], BF16, tag="at_tmp")
                            nc.vector.tensor_mul(at_tmp, u_ps[:DH, :], bc_sb)
                            at_producers.append(nc.sync.dma_start(
                                out=at_sb[g // 2][DH : 2 * DH, :], in_=at_tmp
                            ))

                    # ---- phase C: O-projection for this Lq block ----
                    for lt in range(NB // P):
                        row0 = b * L + c * NB + lt * P
                        for nb in range(D // NB):
                            o_ps = psA.tile([P, NB], F32, tag="acc")
                            for t in range(2):
                                nc.tensor.matmul(
                                    o_ps,
                                    lhsT=at_sb[t][:, lt * P : (lt + 1) * P],
                                    rhs=wo_sb[t][:, nb * NB : (nb + 1) * NB],
                                    start=(t == 0),
                                    stop=(t == 1),
                                )
                            o_sb = opool.tile([P, NB], F32, tag="o")
                            nc.vector.tensor_copy(o_sb, o_ps)
                            nc.sync.dma_start(
                                out=out[row0 : row0 + P, nb * NB : (nb + 1) * NB],
                                in_=o_sb,
                            )
    nc.compile()
    return nc


def kernel(x, Wq, Wk, Wv, Wo, trace=False):
    x = np.ascontiguousarray(np.asarray(x, dtype=np.float32))
    Wq = np.asarray(Wq, dtype=np.float32)
    Wk = np.asarray(Wk, dtype=np.float32)
    Wv = np.asarray(Wv, dtype=np.float32)
    Wo = np.asarray(Wo, dtype=np.float32)

    xT = np.ascontiguousarray(x.reshape(BL, D).T.astype(ml_dtypes.bfloat16))  # [D, BL]
    Wq = Wq.astype(ml_dtypes.bfloat16)
    Wk = Wk.astype(ml_dtypes.bfloat16)
    Wv = Wv.astype(ml_dtypes.bfloat16)
    Wo = Wo.astype(ml_dtypes.bfloat16)

    in_maps = []
    for i in range(HKV):
        qs = slice(i * DQ, (i + 1) * DQ)
        ks = slice(i * DH, (i + 1) * DH)
        in_maps.append(
            {
                "xT": xT,
                "wq": np.ascontiguousarray(Wq[:, qs]),
                "wk": np.ascontiguousarray(np.concatenate([Wk[:, ks], Wk[:, ks]], axis=1)),
                "wv": np.ascontiguousarray(Wv[:, ks]),
                "wo": np.ascontiguousarray(Wo[qs, :]),
            }
        )

    if "nc" not in _CACHED:
        _CACHED["nc"] = build_nc()
    nc = _CACHED["nc"]

    res = run_bass_kernel_spmd(nc, in_maps, list(range(HKV)), trace=trace)
    acc = np.zeros((BL, D), dtype=np.float32)
    for r in res.results:
        acc += r["out"]
    if trace:
        kernel.last_exec_time_ns = res.exec_time_ns
        kernel.last_results = res
    return acc.reshape(B, L, D)



# revision 2
# speedup vs baseline: 7.2431x; 7.2431x over previous
"""GQA kernel for trn2: B=2, L=2048, D=2048, Hq=32, Hkv=8, dh=64.

Sharding: 1 KV head (= 4 contiguous Q heads) per core; Wq/Wk/Wv
column-sharded by head. To minimize host<->device traffic over the
axon/PJRT tunnel (the wall-clock bottleneck), x is uploaded
sequence-sharded (one 512-column slice of xT per core, AllGathered on
device) and the output is produced as disjoint per-core column slices:
the per-core attention outputs (attnT, [256, BL] bf16) are AllGathered
on device and each core contracts the full gathered attnT against its
column shard of Wo, writing out[:, c*256:(c+1)*256] in bf16. The host
just concatenates.

Layout trick: x is transposed on the host (xT: [D, B*L]) so every
on-device matmul has its contraction dim on partitions without any
on-device transposes:
  Q^T[dq, l]  = (Wq_tile).T @ xT        (lhsT=Wq, rhs=xT)
  K^T[dh, l]  = (Wk_tile).T @ xT
  V[l, dh]    = (xT_tile).T @ Wv        (lhsT=xT, rhs=Wv)
  S^T[k, q]   = (K^T_tile).T @ Q^T      (lhsT=K^T, rhs=Q^T)   contract dh=64
  E           = exp(S^T / 8)            (ScalarE, PSUM->SBUF)
  U[0:65, q]  = [V|1].T @ E             (lhsT=V_aug, rhs=E)   contract Lk
                row 64 of U = softmax denominator (ones column trick)
  attnT       = U[:64] * bcast(1/U[64]) (DVE recip + K=1 matmul bcast + mul)
  out[l, mc] += (attnT_all_tile).T @ Wo[:, mc]   (contract full q-dim 2048)
"""

import ml_dtypes
import numpy as np

import concourse.bass as bass
import concourse.bacc as bacc
import concourse.mybir as mybir
from concourse.tile import TileContext
from concourse.bass_utils import run_bass_kernel_spmd

B, L, D = 2, 2048, 2048
HQ, HKV, DH = 32, 8, 64
GQ = HQ // HKV            # 4 q heads per core
DQ = GQ * DH              # 256
BL = B * L                # 4096
P = 128
NB = 512                  # free-dim block
KD = D // P               # 16 contraction tiles over D
LT = L // P               # 16 Lk tiles per batch
NBLK = L // NB            # 4 Lq blocks per batch
NCORES = HKV              # 8
SCALE = 1.0 / 8.0         # 1/sqrt(dh)

F32 = mybir.dt.float32
BF16 = mybir.dt.bfloat16
AF = mybir.ActivationFunctionType

_CACHED = {}


def build_nc():
    nc = bacc.Bacc()
    xs = nc.declare_dram_parameter("xs", [D, NB], BF16, isOutput=False)
    wq = nc.declare_dram_parameter("wq", [D, DQ], BF16, isOutput=False)
    wk = nc.declare_dram_parameter("wk", [D, DH], BF16, isOutput=False)
    wv = nc.declare_dram_parameter("wv", [D, DH], BF16, isOutput=False)
    wo = nc.declare_dram_parameter("wo", [D, DQ], BF16, isOutput=False)
    out = nc.declare_dram_parameter("out", [BL, DQ], BF16, isOutput=True)

    groups = [list(range(NCORES))]

    with TileContext(nc) as tc:
        with (
            tc.tile_pool(name="dram", bufs=1, space="DRAM") as dram,
            tc.tile_pool(name="wpool", bufs=1) as wpool,
            tc.tile_pool(name="xpool", bufs=3) as xpool,
            tc.tile_pool(name="qtpool", bufs=3) as qtpool,
            tc.tile_pool(name="ktpool", bufs=2) as ktpool,
            tc.tile_pool(name="vpool", bufs=34) as vpool,
            tc.tile_pool(name="epool", bufs=20) as epool,
            tc.tile_pool(name="atpool", bufs=2) as atpool,
            tc.tile_pool(name="atgpool", bufs=3) as atgpool,
            tc.tile_pool(name="opool", bufs=3) as opool,
            tc.tile_pool(name="bcpool", bufs=2) as bcpool,
            tc.tile_pool(name="rpool", bufs=4) as rpool,
            tc.tile_pool(name="psA", bufs=2, space="PSUM") as psA,
            tc.tile_pool(name="psS", bufs=4, space="PSUM") as psS,
            tc.tile_pool(name="psU", bufs=2, space="PSUM") as psU,
        ):
            # ---- gather the sequence-sharded xT across cores ----
            xin = dram.tile([D, NB], BF16, tag="xin")
            xg = dram.tile([NCORES * D, NB], BF16, tag="xg")
            nc.sync.dma_start(out=xin, in_=xs[:, :])
            nc.gpsimd.collective_compute(
                "AllGather",
                mybir.AluOpType.bypass,
                replica_groups=groups,
                ins=[xin.opt()],
                outs=[xg.opt()],
            )
            # shard s of xg = xT[:, s*NB:(s+1)*NB]
            xg_r = xg.rearrange("(s k p) n -> p (s k) n", s=NCORES, p=P)

            # attnT staging (collective in/out)
            at_in = dram.tile([DQ, BL], BF16, tag="at_in")
            at_all = dram.tile([NCORES * DQ, BL], BF16, tag="at_all")

            # ---- persistent weights ----
            wq_sb = wpool.tile([P, KD, DQ], BF16, tag="wq")
            nc.sync.dma_start(out=wq_sb, in_=wq.rearrange("(k p) m -> p k m", p=P))
            wk_sb = wpool.tile([P, KD, 2 * DH], BF16, tag="wk")
            nc.sync.dma_start(
                out=wk_sb[:, :, 0:DH], in_=wk.rearrange("(k p) m -> p k m", p=P)
            )
            nc.sync.dma_start(
                out=wk_sb[:, :, DH : 2 * DH],
                in_=wk.rearrange("(k p) m -> p k m", p=P),
            )
            wv_sb = wpool.tile([P, KD, DH], BF16, tag="wv")
            nc.sync.dma_start(out=wv_sb, in_=wv.rearrange("(k p) m -> p k m", p=P))
            wo_sb = wpool.tile([P, KD, DQ], BF16, tag="wo")
            nc.sync.dma_start(out=wo_sb, in_=wo.rearrange("(k p) m -> p k m", p=P))
            ones_sb = wpool.tile([1, DH], BF16, tag="ones")
            nc.vector.memset(ones_sb, 1.0)

            for b in range(B):
                # ---------- phase A: projections for batch b ----------
                qt_sb = [qtpool.tile([P, L], BF16, tag="qt", name=f"qt_sb{t}") for t in range(2)]
                kt_sb = ktpool.tile([P, L], BF16, tag="kt")
                v_sb = [vpool.tile([P, DH + 1], BF16, tag="v", name=f"v_sb{k}") for k in range(LT)]

                for c in range(NBLK):
                    s = b * NBLK + c  # global 512-col block == gather shard
                    xt_all = xpool.tile([P, KD, NB], BF16, tag="xt")
                    nc.sync.dma_start(
                        out=xt_all, in_=xg_r[:, s * KD : (s + 1) * KD, :]
                    )

                    # Q^T (two 128-row dq tiles)
                    for t in range(2):
                        q_ps = psA.tile([P, NB], F32, tag="acc")
                        for k in range(KD):
                            nc.tensor.matmul(
                                q_ps,
                                lhsT=wq_sb[:, k, t * P : (t + 1) * P],
                                rhs=xt_all[:, k, :],
                                start=(k == 0),
                                stop=(k == KD - 1),
                            )
                        nc.vector.tensor_copy(qt_sb[t][:, c * NB : (c + 1) * NB], q_ps)
                    # K^T
                    k_ps = psA.tile([P, NB], F32, tag="acc")
                    for k in range(KD):
                        nc.tensor.matmul(
                            k_ps,
                            lhsT=wk_sb[:, k, :],
                            rhs=xt_all[:, k, :],
                            start=(k == 0),
                            stop=(k == KD - 1),
                        )
                    nc.vector.tensor_copy(kt_sb[:, c * NB : (c + 1) * NB], k_ps)
                    # V (natural, Lk-major) + ones column
                    for j in range(NB // P):
                        lk = c * (NB // P) + j
                        v_ps = psA.tile([P, DH], F32, tag="acc")
                        for k in range(KD):
                            nc.tensor.matmul(
                                v_ps,
                                lhsT=xt_all[:, k, j * P : (j + 1) * P],
                                rhs=wv_sb[:, k, :],
                                start=(k == 0),
                                stop=(k == KD - 1),
                            )
                        nc.vector.tensor_copy(v_sb[lk][:, :DH], v_ps)
                        nc.vector.memset(v_sb[lk][:, DH : DH + 1], 1.0)

                # ---------- phase B per Lq block ----------
                for c in range(NBLK):
                    at_sb = [atpool.tile([P, NB], BF16, tag="at", name=f"at_sb{t}") for t in range(2)]
                    for g in range(GQ):
                        qg = qt_sb[g // 2][
                            (g % 2) * DH : (g % 2) * DH + DH, c * NB : (c + 1) * NB
                        ]
                        # S^T tiles + exp; interleave PV to keep PE/ACT in step
                        e_sb = []
                        u_ps = psU.tile([P, NB], F32, tag="u")

                        h0 = (g % 2) * DH

                        def qk_step(k):
                            sT = psS.tile([P, NB], F32, tag="sT")
                            nc.tensor.matmul(
                                sT,
                                lhsT=kt_sb[h0 : h0 + DH, k * P : (k + 1) * P],
                                rhs=qg,
                                start=True,
                                stop=True,
                            )
                            e = epool.tile([P, NB], BF16, tag="e")
                            nc.scalar.activation(e, sT, AF.Exp, scale=SCALE)
                            e_sb.append(e)

                        def pv_step(k):
                            nc.tensor.matmul(
                                u_ps[: DH + 1, :],
                                lhsT=v_sb[k][:, :],
                                rhs=e_sb[k],
                                start=(k == 0),
                                stop=(k == LT - 1),
                            )

                        for k in range(4):
                            qk_step(k)
                        for k in range(4, LT):
                            qk_step(k)
                            pv_step(k - 4)
                        for k in range(LT - 4, LT):
                            pv_step(k)

                        # normalize: attnT = U[:64] * bcast(1 / U[64])
                        recip = rpool.tile([1, NB], BF16, tag="r")
                        with nc.allow_low_precision(reason="f32r is fp32-width"):
                            nc.vector.reciprocal(recip, u_ps[DH : DH + 1, :])
                        bc_ps = psS.tile([DH, NB], F32, tag="sT")
                        nc.tensor.matmul(
                            bc_ps, lhsT=ones_sb, rhs=recip, start=True, stop=True
                        )
                        bc_sb = bcpool.tile([DH, NB], F32, tag="bc")
                        nc.vector.tensor_copy(bc_sb, bc_ps)
                        if g % 2 == 0:
                            nc.vector.tensor_mul(
                                at_sb[g // 2][:DH, :], u_ps[:DH, :], bc_sb
                            )
                        else:
                            at_tmp = rpool.tile([DH, NB], BF16, tag="at_tmp")
                            nc.vector.tensor_mul(at_tmp, u_ps[:DH, :], bc_sb)
                            nc.sync.dma_start(
                                out=at_sb[g // 2][DH : 2 * DH, :], in_=at_tmp
                            )

                    # stage attnT for the cross-core gather
                    c0 = b * L + c * NB
                    for t in range(2):
                        nc.sync.dma_start(
                            out=at_in[t * P : (t + 1) * P, c0 : c0 + NB],
                            in_=at_sb[t],
                        )

            # ---------- gather attnT across cores ----------
            nc.gpsimd.collective_compute(
                "AllGather",
                mybir.AluOpType.bypass,
                replica_groups=groups,
                ins=[at_in.opt()],
                outs=[at_all.opt()],
            )
            # global q-dim chunk j = rows j*128..(j+1)*128 of at_all
            at_r = at_all.rearrange("(k p) l -> p k l", p=P)  # [128, 16, BL]

            # ---------- phase C: disjoint output column slice ----------
            for lb in range(BL // P):
                atg = atgpool.tile([P, KD, P], BF16, tag="atg")
                nc.sync.dma_start(out=atg, in_=at_r[:, :, lb * P : (lb + 1) * P])
                o_ps = psA.tile([P, DQ], F32, tag="acc")
                for k in range(KD):
                    nc.tensor.matmul(
                        o_ps,
                        lhsT=atg[:, k, :],
                        rhs=wo_sb[:, k, :],
                        start=(k == 0),
                        stop=(k == KD - 1),
                    )
                o_sb = opool.tile([P, DQ], BF16, tag="o")
                nc.vector.tensor_copy(o_sb, o_ps)
                nc.sync.dma_start(out=out[lb * P : (lb + 1) * P, :], in_=o_sb)
    nc.compile()
    return nc


def kernel(x, Wq, Wk, Wv, Wo, trace=False):
    x = np.ascontiguousarray(np.asarray(x, dtype=np.float32))
    Wq = np.asarray(Wq, dtype=np.float32).astype(ml_dtypes.bfloat16)
    Wk = np.asarray(Wk, dtype=np.float32).astype(ml_dtypes.bfloat16)
    Wv = np.asarray(Wv, dtype=np.float32).astype(ml_dtypes.bfloat16)
    Wo = np.asarray(Wo, dtype=np.float32).astype(ml_dtypes.bfloat16)

    xT = np.ascontiguousarray(x.reshape(BL, D).T.astype(ml_dtypes.bfloat16))  # [D, BL]

    in_maps = []
    for i in range(NCORES):
        qs = slice(i * DQ, (i + 1) * DQ)
        ks = slice(i * DH, (i + 1) * DH)
        in_maps.append(
            {
                "xs": np.ascontiguousarray(xT[:, i * NB : (i + 1) * NB]),
                "wq": np.ascontiguousarray(Wq[:, qs]),
                "wk": np.ascontiguousarray(Wk[:, ks]),
                "wv": np.ascontiguousarray(Wv[:, ks]),
                "wo": np.ascontiguousarray(Wo[:, qs]),
            }
        )

    if "nc" not in _CACHED:
        _CACHED["nc"] = build_nc()
    nc = _CACHED["nc"]

    res = run_bass_kernel_spmd(nc, in_maps, list(range(NCORES)), trace=trace)
    acc = np.concatenate(
        [np.asarray(r["out"]) for r in res.results], axis=1
    ).astype(np.float32)
    if trace:
        kernel.last_exec_time_ns = res.exec_time_ns
        kernel.last_results = res
    return acc.reshape(B, L, D)


# revision 3
# speedup vs baseline: 7.8624x; 1.0855x over previous
"""GQA kernel for trn2: B=2, L=2048, D=2048, Hq=32, Hkv=8, dh=64.

Sharding: 1 KV head (= 4 contiguous Q heads) per core; Wq/Wk/Wv
column-sharded by head. To minimize host<->device traffic over the
axon/PJRT tunnel (the wall-clock bottleneck), x is uploaded
sequence-sharded (one 512-column slice of xT per core, AllGathered on
device) and the output is produced as disjoint per-core column slices:
the per-core attention outputs (attnT, [256, BL] bf16) are AllGathered
on device and each core contracts the full gathered attnT against its
column shard of Wo, writing out[:, c*256:(c+1)*256] in bf16. The host
just concatenates.

Layout trick: x is transposed on the host (xT: [D, B*L]) so every
on-device matmul has its contraction dim on partitions without any
on-device transposes:
  Q^T[dq, l]  = (Wq_tile).T @ xT        (lhsT=Wq, rhs=xT)
  K^T[dh, l]  = (Wk_tile).T @ xT
  V[l, dh]    = (xT_tile).T @ Wv        (lhsT=xT, rhs=Wv)
  S^T[k, q]   = (K^T_tile).T @ Q^T      (lhsT=K^T, rhs=Q^T)   contract dh=64
  E           = exp(S^T / 8)            (ScalarE, PSUM->SBUF)
  U[0:65, q]  = [V|1].T @ E             (lhsT=V_aug, rhs=E)   contract Lk
                row 64 of U = softmax denominator (ones column trick)
  attnT       = U[:64] * bcast(1/U[64]) (DVE recip + K=1 matmul bcast + mul)
  out[l, mc] += (attnT_all_tile).T @ Wo[:, mc]   (contract full q-dim 2048)
"""

import os
import tempfile

import ml_dtypes
import numpy as np

import jax

# Persistent compilation cache: run_bass_kernel_spmd re-jits per call; a
# disk hit skips the client-side BIR reprocessing (~0.3s/call).
jax.config.update(
    "jax_compilation_cache_dir",
    os.path.join(tempfile.gettempdir(), "jax_comp_cache"),
)
jax.config.update("jax_persistent_cache_min_entry_size_bytes", -1)
jax.config.update("jax_persistent_cache_min_compile_time_secs", 0)

import concourse.bass as bass
import concourse.bacc as bacc
import concourse.mybir as mybir
from concourse.tile import TileContext
from concourse.bass_utils import run_bass_kernel_spmd

B, L, D = 2, 2048, 2048
HQ, HKV, DH = 32, 8, 64
GQ = HQ // HKV            # 4 q heads per core
DQ = GQ * DH              # 256
BL = B * L                # 4096
P = 128
NB = 512                  # free-dim block
KD = D // P               # 16 contraction tiles over D
LT = L // P               # 16 Lk tiles per batch
NBLK = L // NB            # 4 Lq blocks per batch
NCORES = HKV              # 8
SCALE = 1.0 / 8.0         # 1/sqrt(dh)

F32 = mybir.dt.float32
BF16 = mybir.dt.bfloat16
AF = mybir.ActivationFunctionType

_CACHED = {}


def build_nc():
    nc = bacc.Bacc()
    xs = nc.declare_dram_parameter("xs", [D, NB], BF16, isOutput=False)
    wq = nc.declare_dram_parameter("wq", [D, DQ], BF16, isOutput=False)
    wk = nc.declare_dram_parameter("wk", [D, DH], BF16, isOutput=False)
    wv = nc.declare_dram_parameter("wv", [D, DH], BF16, isOutput=False)
    wo = nc.declare_dram_parameter("wo", [D, DQ], BF16, isOutput=False)
    out = nc.declare_dram_parameter("out", [BL, DQ], BF16, isOutput=True)

    groups = [list(range(NCORES))]

    with TileContext(nc) as tc:
        with (
            tc.tile_pool(name="dram", bufs=1, space="DRAM") as dram,
            tc.tile_pool(name="wpool", bufs=1) as wpool,
            tc.tile_pool(name="xpool", bufs=3) as xpool,
            tc.tile_pool(name="qtpool", bufs=3) as qtpool,
            tc.tile_pool(name="ktpool", bufs=2) as ktpool,
            tc.tile_pool(name="vpool", bufs=34) as vpool,
            tc.tile_pool(name="epool", bufs=20) as epool,
            tc.tile_pool(name="atpool", bufs=2) as atpool,
            tc.tile_pool(name="atgpool", bufs=3) as atgpool,
            tc.tile_pool(name="opool", bufs=3) as opool,
            tc.tile_pool(name="bcpool", bufs=2) as bcpool,
            tc.tile_pool(name="rpool", bufs=4) as rpool,
            tc.tile_pool(name="psA", bufs=2, space="PSUM") as psA,
            tc.tile_pool(name="psS", bufs=4, space="PSUM") as psS,
            tc.tile_pool(name="psU", bufs=2, space="PSUM") as psU,
        ):
            # ---- gather the sequence-sharded xT across cores ----
            xin = dram.tile([D, NB], BF16, tag="xin")
            xg = dram.tile([NCORES * D, NB], BF16, tag="xg")
            nc.sync.dma_start(out=xin, in_=xs[:, :])
            nc.gpsimd.collective_compute(
                "AllGather",
                mybir.AluOpType.bypass,
                replica_groups=groups,
                ins=[xin.opt()],
                outs=[xg.opt()],
            )
            # shard s of xg = xT[:, s*NB:(s+1)*NB]
            xg_r = xg.rearrange("(s k p) n -> p (s k) n", s=NCORES, p=P)

            # attnT staging (collective in/out)
            at_in = dram.tile([DQ, BL], BF16, tag="at_in")
            at_all = dram.tile([NCORES * DQ, BL], BF16, tag="at_all")

            # ---- persistent weights ----
            wq_sb = wpool.tile([P, KD, DQ], BF16, tag="wq")
            nc.sync.dma_start(out=wq_sb, in_=wq.rearrange("(k p) m -> p k m", p=P))
            wk_sb = wpool.tile([P, KD, 2 * DH], BF16, tag="wk")
            nc.sync.dma_start(
                out=wk_sb[:, :, 0:DH], in_=wk.rearrange("(k p) m -> p k m", p=P)
            )
            nc.sync.dma_start(
                out=wk_sb[:, :, DH : 2 * DH],
                in_=wk.rearrange("(k p) m -> p k m", p=P),
            )
            wv_sb = wpool.tile([P, KD, DH], BF16, tag="wv")
            nc.sync.dma_start(out=wv_sb, in_=wv.rearrange("(k p) m -> p k m", p=P))
            wo_sb = wpool.tile([P, KD, DQ], BF16, tag="wo")
            nc.sync.dma_start(out=wo_sb, in_=wo.rearrange("(k p) m -> p k m", p=P))
            ones_sb = wpool.tile([1, DH], BF16, tag="ones")
            nc.vector.memset(ones_sb, 1.0)

            for b in range(B):
                # ---------- phase A: projections for batch b ----------
                qt_sb = [qtpool.tile([P, L], BF16, tag="qt", name=f"qt_sb{t}") for t in range(2)]
                kt_sb = ktpool.tile([P, L], BF16, tag="kt")
                v_sb = [vpool.tile([P, DH + 1], BF16, tag="v", name=f"v_sb{k}") for k in range(LT)]

                for c in range(NBLK):
                    s = b * NBLK + c  # global 512-col block == gather shard
                    xt_all = xpool.tile([P, KD, NB], BF16, tag="xt")
                    nc.sync.dma_start(
                        out=xt_all, in_=xg_r[:, s * KD : (s + 1) * KD, :]
                    )

                    # Q^T (two 128-row dq tiles)
                    for t in range(2):
                        q_ps = psA.tile([P, NB], F32, tag="acc")
                        for k in range(KD):
                            nc.tensor.matmul(
                                q_ps,
                                lhsT=wq_sb[:, k, t * P : (t + 1) * P],
                                rhs=xt_all[:, k, :],
                                start=(k == 0),
                                stop=(k == KD - 1),
                            )
                        nc.vector.tensor_copy(qt_sb[t][:, c * NB : (c + 1) * NB], q_ps)
                    # K^T
                    k_ps = psA.tile([P, NB], F32, tag="acc")
                    for k in range(KD):
                        nc.tensor.matmul(
                            k_ps,
                            lhsT=wk_sb[:, k, :],
                            rhs=xt_all[:, k, :],
                            start=(k == 0),
                            stop=(k == KD - 1),
                        )
                    nc.vector.tensor_copy(kt_sb[:, c * NB : (c + 1) * NB], k_ps)
                    # V (natural, Lk-major) + ones column
                    for j in range(NB // P):
                        lk = c * (NB // P) + j
                        v_ps = psA.tile([P, DH], F32, tag="acc")
                        for k in range(KD):
                            nc.tensor.matmul(
                                v_ps,
                                lhsT=xt_all[:, k, j * P : (j + 1) * P],
                                rhs=wv_sb[:, k, :],
                                start=(k == 0),
                                stop=(k == KD - 1),
                            )
                        nc.vector.tensor_copy(v_sb[lk][:, :DH], v_ps)
                        nc.vector.memset(v_sb[lk][:, DH : DH + 1], 1.0)

                # ---------- phase B per Lq block ----------
                for c in range(NBLK):
                    at_sb = [atpool.tile([P, NB], BF16, tag="at", name=f"at_sb{t}") for t in range(2)]
                    for g in range(GQ):
                        qg = qt_sb[g // 2][
                            (g % 2) * DH : (g % 2) * DH + DH, c * NB : (c + 1) * NB
                        ]
                        # S^T tiles + exp; interleave PV to keep PE/ACT in step
                        e_sb = []
                        u_ps = psU.tile([P, NB], F32, tag="u")

                        h0 = (g % 2) * DH

                        def qk_step(k):
                            sT = psS.tile([P, NB], F32, tag="sT")
                            nc.tensor.matmul(
                                sT,
                                lhsT=kt_sb[h0 : h0 + DH, k * P : (k + 1) * P],
                                rhs=qg,
                                start=True,
                                stop=True,
                            )
                            e = epool.tile([P, NB], BF16, tag="e")
                            nc.scalar.activation(e, sT, AF.Exp, scale=SCALE)
                            e_sb.append(e)

                        def pv_step(k):
                            nc.tensor.matmul(
                                u_ps[: DH + 1, :],
                                lhsT=v_sb[k][:, :],
                                rhs=e_sb[k],
                                start=(k == 0),
                                stop=(k == LT - 1),
                            )

                        for k in range(4):
                            qk_step(k)
                        for k in range(4, LT):
                            qk_step(k)
                            pv_step(k - 4)
                        for k in range(LT - 4, LT):
                            pv_step(k)

                        # normalize: attnT = U[:64] * bcast(1 / U[64])
                        recip = rpool.tile([1, NB], BF16, tag="r")
                        with nc.allow_low_precision(reason="f32r is fp32-width"):
                            nc.vector.reciprocal(recip, u_ps[DH : DH + 1, :])
                        bc_ps = psS.tile([DH, NB], F32, tag="sT")
                        nc.tensor.matmul(
                            bc_ps, lhsT=ones_sb, rhs=recip, start=True, stop=True
                        )
                        bc_sb = bcpool.tile([DH, NB], F32, tag="bc")
                        nc.vector.tensor_copy(bc_sb, bc_ps)
                        if g % 2 == 0:
                            nc.vector.tensor_mul(
                                at_sb[g // 2][:DH, :], u_ps[:DH, :], bc_sb
                            )
                        else:
                            at_tmp = rpool.tile([DH, NB], BF16, tag="at_tmp")
                            nc.vector.tensor_mul(at_tmp, u_ps[:DH, :], bc_sb)
                            nc.sync.dma_start(
                                out=at_sb[g // 2][DH : 2 * DH, :], in_=at_tmp
                            )

                    # stage attnT for the cross-core gather
                    c0 = b * L + c * NB
                    for t in range(2):
                        nc.sync.dma_start(
                            out=at_in[t * P : (t + 1) * P, c0 : c0 + NB],
                            in_=at_sb[t],
                        )

            # ---------- gather attnT across cores ----------
            nc.gpsimd.collective_compute(
                "AllGather",
                mybir.AluOpType.bypass,
                replica_groups=groups,
                ins=[at_in.opt()],
                outs=[at_all.opt()],
            )
            # global q-dim chunk j = rows j*128..(j+1)*128 of at_all
            at_r = at_all.rearrange("(k p) l -> p k l", p=P)  # [128, 16, BL]

            # ---------- phase C: disjoint output column slice ----------
            for lb in range(BL // P):
                atg = atgpool.tile([P, KD, P], BF16, tag="atg")
                nc.sync.dma_start(out=atg, in_=at_r[:, :, lb * P : (lb + 1) * P])
                o_ps = psA.tile([P, DQ], F32, tag="acc")
                for k in range(KD):
                    nc.tensor.matmul(
                        o_ps,
                        lhsT=atg[:, k, :],
                        rhs=wo_sb[:, k, :],
                        start=(k == 0),
                        stop=(k == KD - 1),
                    )
                o_sb = opool.tile([P, DQ], BF16, tag="o")
                nc.vector.tensor_copy(o_sb, o_ps)
                nc.sync.dma_start(out=out[lb * P : (lb + 1) * P, :], in_=o_sb)
    nc.compile()
    return nc


def kernel(x, Wq, Wk, Wv, Wo, trace=False):
    x = np.ascontiguousarray(np.asarray(x, dtype=np.float32))
    Wq = np.asarray(Wq, dtype=np.float32).astype(ml_dtypes.bfloat16)
    Wk = np.asarray(Wk, dtype=np.float32).astype(ml_dtypes.bfloat16)
    Wv = np.asarray(Wv, dtype=np.float32).astype(ml_dtypes.bfloat16)
    Wo = np.asarray(Wo, dtype=np.float32).astype(ml_dtypes.bfloat16)

    xT = np.ascontiguousarray(x.reshape(BL, D).T.astype(ml_dtypes.bfloat16))  # [D, BL]

    in_maps = []
    for i in range(NCORES):
        qs = slice(i * DQ, (i + 1) * DQ)
        ks = slice(i * DH, (i + 1) * DH)
        in_maps.append(
            {
                "xs": np.ascontiguousarray(xT[:, i * NB : (i + 1) * NB]),
                "wq": np.ascontiguousarray(Wq[:, qs]),
                "wk": np.ascontiguousarray(Wk[:, ks]),
                "wv": np.ascontiguousarray(Wv[:, ks]),
                "wo": np.ascontiguousarray(Wo[:, qs]),
            }
        )

    if "nc" not in _CACHED:
        _CACHED["nc"] = build_nc()
    nc = _CACHED["nc"]

    res = run_bass_kernel_spmd(nc, in_maps, list(range(NCORES)), trace=trace)
    acc = np.concatenate(
        [np.asarray(r["out"]) for r in res.results], axis=1
    ).astype(np.float32)
    if trace:
        kernel.last_exec_time_ns = res.exec_time_ns
        kernel.last_results = res
    return acc.reshape(B, L, D)


# revision 6
# speedup vs baseline: 8.8854x; 1.1301x over previous
"""GQA kernel for trn2: B=2, L=2048, D=2048, Hq=32, Hkv=8, dh=64.

Sharding: 1 KV head (= 4 contiguous Q heads) per core; Wq/Wk/Wv
column-sharded by head. To minimize host<->device traffic over the
axon/PJRT tunnel (the wall-clock bottleneck), x is uploaded
sequence-sharded (one 512-column slice of xT per core, AllGathered on
device) and the output is produced as disjoint per-core column slices:
the per-core attention outputs (attnT, [256, BL] bf16) are AllGathered
on device and each core contracts the full gathered attnT against its
column shard of Wo, writing out[:, c*256:(c+1)*256] in bf16. The host
just concatenates.

Layout trick: x is transposed on the host (xT: [D, B*L]) so every
on-device matmul has its contraction dim on partitions without any
on-device transposes:
  Q^T[dq, l]  = (Wq_tile).T @ xT        (lhsT=Wq, rhs=xT)
  K^T[dh, l]  = (Wk_tile).T @ xT
  V[l, dh]    = (xT_tile).T @ Wv        (lhsT=xT, rhs=Wv)
  S^T[k, q]   = (K^T_tile).T @ Q^T      (lhsT=K^T, rhs=Q^T)   contract dh=64
  E           = exp(S^T / 8)            (ScalarE, PSUM->SBUF)
  U[0:65, q]  = [V|1].T @ E             (lhsT=V_aug, rhs=E)   contract Lk
                row 64 of U = softmax denominator (ones column trick)
  attnT       = U[:64] * bcast(1/U[64]) (DVE recip + K=1 matmul bcast + mul)
  out[l, mc] += (attnT_all_tile).T @ Wo[:, mc]   (contract full q-dim 2048)
"""

import os
import tempfile

import ml_dtypes
import numpy as np

import jax

# Persistent compilation cache: run_bass_kernel_spmd re-jits per call; a
# disk hit skips the client-side BIR reprocessing (~0.3s/call).
jax.config.update(
    "jax_compilation_cache_dir",
    os.path.join(tempfile.gettempdir(), "jax_comp_cache"),
)
jax.config.update("jax_persistent_cache_min_entry_size_bytes", -1)
jax.config.update("jax_persistent_cache_min_compile_time_secs", 0)

import concourse.bass as bass
import concourse.bacc as bacc
import concourse.mybir as mybir
from concourse.tile import TileContext
from concourse.bass_utils import run_bass_kernel_spmd

B, L, D = 2, 2048, 2048
HQ, HKV, DH = 32, 8, 64
GQ = HQ // HKV            # 4 q heads per core
DQ = GQ * DH              # 256
BL = B * L                # 4096
P = 128
NB = 512                  # free-dim block
KD = D // P               # 16 contraction tiles over D
LT = L // P               # 16 Lk tiles per batch
NBLK = L // NB            # 4 Lq blocks per batch
NCORES = HKV              # 8
SCALE = 1.0 / 8.0         # 1/sqrt(dh)

F32 = mybir.dt.float32
BF16 = mybir.dt.bfloat16
AF = mybir.ActivationFunctionType

_CACHED = {}


def build_nc():
    nc = bacc.Bacc()
    xs = nc.declare_dram_parameter("xs", [NB, D], BF16, isOutput=False)
    wq = nc.declare_dram_parameter("wq", [D, DQ], BF16, isOutput=False)
    wk = nc.declare_dram_parameter("wk", [D, DH], BF16, isOutput=False)
    wv = nc.declare_dram_parameter("wv", [D, DH], BF16, isOutput=False)
    wo = nc.declare_dram_parameter("wo", [D, DQ], BF16, isOutput=False)
    out = nc.declare_dram_parameter("out", [BL, DQ], BF16, isOutput=True)

    groups = [list(range(NCORES))]

    with TileContext(nc) as tc:
        with (
            tc.tile_pool(name="dram", bufs=1, space="DRAM") as dram,
            tc.tile_pool(name="wpool", bufs=1) as wpool,
            tc.tile_pool(name="xpool", bufs=3) as xpool,
            tc.tile_pool(name="qtpool", bufs=3) as qtpool,
            tc.tile_pool(name="ktpool", bufs=2) as ktpool,
            tc.tile_pool(name="vpool", bufs=34) as vpool,
            tc.tile_pool(name="epool", bufs=20) as epool,
            tc.tile_pool(name="atpool", bufs=2) as atpool,
            tc.tile_pool(name="atgpool", bufs=3) as atgpool,
            tc.tile_pool(name="opool", bufs=3) as opool,
            tc.tile_pool(name="bcpool", bufs=2) as bcpool,
            tc.tile_pool(name="rpool", bufs=4) as rpool,
            tc.tile_pool(name="psA", bufs=2, space="PSUM") as psA,
            tc.tile_pool(name="psS", bufs=4, space="PSUM") as psS,
            tc.tile_pool(name="psU", bufs=2, space="PSUM") as psU,
        ):
            # ---- transpose the natural-layout x shard on device (XBAR),
            # then gather the sequence-sharded xT across cores ----
            xin = dram.tile([D, NB], BF16, tag="xin")
            xg = dram.tile([NCORES * D, NB], BF16, tag="xg")
            with tc.tile_pool(name="trpool", bufs=4) as trpool:
                for k in range(KD):
                    tr = trpool.tile([P, NB], BF16, tag="tr", name=f"tr{k}")
                    nc.sync.dma_start_transpose(
                        out=tr, in_=xs[:, k * P : (k + 1) * P]
                    )
                    nc.sync.dma_start(out=xin[k * P : (k + 1) * P, :], in_=tr)
            nc.gpsimd.collective_compute(
                "AllGather",
                mybir.AluOpType.bypass,
                replica_groups=groups,
                ins=[xin.opt()],
                outs=[xg.opt()],
            )
            # shard s of xg = xT[:, s*NB:(s+1)*NB]
            xg_r = xg.rearrange("(s k p) n -> p (s k) n", s=NCORES, p=P)

            # attnT staging (collective in/out)
            at_in = dram.tile([DQ, BL], BF16, tag="at_in")
            at_all = dram.tile([NCORES * DQ, BL], BF16, tag="at_all")

            # ---- persistent weights ----
            wq_sb = wpool.tile([P, KD, DQ], BF16, tag="wq")
            nc.sync.dma_start(out=wq_sb, in_=wq.rearrange("(k p) m -> p k m", p=P))
            wk_sb = wpool.tile([P, KD, 2 * DH], BF16, tag="wk")
            nc.sync.dma_start(
                out=wk_sb[:, :, 0:DH], in_=wk.rearrange("(k p) m -> p k m", p=P)
            )
            nc.sync.dma_start(
                out=wk_sb[:, :, DH : 2 * DH],
                in_=wk.rearrange("(k p) m -> p k m", p=P),
            )
            wv_sb = wpool.tile([P, KD, DH], BF16, tag="wv")
            nc.sync.dma_start(out=wv_sb, in_=wv.rearrange("(k p) m -> p k m", p=P))
            wo_sb = wpool.tile([P, KD, DQ], BF16, tag="wo")
            nc.sync.dma_start(out=wo_sb, in_=wo.rearrange("(k p) m -> p k m", p=P))
            ones_sb = wpool.tile([1, DH], BF16, tag="ones")
            nc.vector.memset(ones_sb, 1.0)

            for b in range(B):
                # ---------- phase A: projections for batch b ----------
                qt_sb = [qtpool.tile([P, L], BF16, tag="qt", name=f"qt_sb{t}") for t in range(2)]
                kt_sb = ktpool.tile([P, L], BF16, tag="kt")
                v_sb = [vpool.tile([P, DH + 1], BF16, tag="v", name=f"v_sb{k}") for k in range(LT)]

                for c in range(NBLK):
                    s = b * NBLK + c  # global 512-col block == gather shard
                    xt_all = xpool.tile([P, KD, NB], BF16, tag="xt")
                    nc.sync.dma_start(
                        out=xt_all, in_=xg_r[:, s * KD : (s + 1) * KD, :]
                    )

                    # Q^T (two 128-row dq tiles)
                    for t in range(2):
                        q_ps = psA.tile([P, NB], F32, tag="acc")
                        for k in range(KD):
                            nc.tensor.matmul(
                                q_ps,
                                lhsT=wq_sb[:, k, t * P : (t + 1) * P],
                                rhs=xt_all[:, k, :],
                                start=(k == 0),
                                stop=(k == KD - 1),
                            )
                        nc.vector.tensor_copy(qt_sb[t][:, c * NB : (c + 1) * NB], q_ps)
                    # K^T
                    k_ps = psA.tile([P, NB], F32, tag="acc")
                    for k in range(KD):
                        nc.tensor.matmul(
                            k_ps,
                            lhsT=wk_sb[:, k, :],
                            rhs=xt_all[:, k, :],
                            start=(k == 0),
                            stop=(k == KD - 1),
                        )
                    nc.vector.tensor_copy(kt_sb[:, c * NB : (c + 1) * NB], k_ps)
                    # V (natural, Lk-major) + ones column
                    for j in range(NB // P):
                        lk = c * (NB // P) + j
                        v_ps = psA.tile([P, DH], F32, tag="acc")
                        for k in range(KD):
                            nc.tensor.matmul(
                                v_ps,
                                lhsT=xt_all[:, k, j * P : (j + 1) * P],
                                rhs=wv_sb[:, k, :],
                                start=(k == 0),
                                stop=(k == KD - 1),
                            )
                        nc.vector.tensor_copy(v_sb[lk][:, :DH], v_ps)
                        nc.vector.memset(v_sb[lk][:, DH : DH + 1], 1.0)

                # ---------- phase B per Lq block ----------
                for c in range(NBLK):
                    at_sb = [atpool.tile([P, NB], BF16, tag="at", name=f"at_sb{t}") for t in range(2)]
                    for g in range(GQ):
                        qg = qt_sb[g // 2][
                            (g % 2) * DH : (g % 2) * DH + DH, c * NB : (c + 1) * NB
                        ]
                        # S^T tiles + exp; interleave PV to keep PE/ACT in step
                        e_sb = []
                        u_ps = psU.tile([P, NB], F32, tag="u")

                        h0 = (g % 2) * DH

                        def qk_step(k):
                            sT = psS.tile([P, NB], F32, tag="sT")
                            nc.tensor.matmul(
                                sT,
                                lhsT=kt_sb[h0 : h0 + DH, k * P : (k + 1) * P],
                                rhs=qg,
                                start=True,
                                stop=True,
                            )
                            e = epool.tile([P, NB], BF16, tag="e")
                            nc.scalar.activation(e, sT, AF.Exp, scale=SCALE)
                            e_sb.append(e)

                        def pv_step(k):
                            nc.tensor.matmul(
                                u_ps[: DH + 1, :],
                                lhsT=v_sb[k][:, :],
                                rhs=e_sb[k],
                                start=(k == 0),
                                stop=(k == LT - 1),
                            )

                        for k in range(4):
                            qk_step(k)
                        for k in range(4, LT):
                            qk_step(k)
                            pv_step(k - 4)
                        for k in range(LT - 4, LT):
                            pv_step(k)

                        # normalize: attnT = U[:64] * bcast(1 / U[64])
                        recip = rpool.tile([1, NB], BF16, tag="r")
                        with nc.allow_low_precision(reason="f32r is fp32-width"):
                            nc.vector.reciprocal(recip, u_ps[DH : DH + 1, :])
                        bc_ps = psS.tile([DH, NB], F32, tag="sT")
                        nc.tensor.matmul(
                            bc_ps, lhsT=ones_sb, rhs=recip, start=True, stop=True
                        )
                        bc_sb = bcpool.tile([DH, NB], F32, tag="bc")
                        nc.vector.tensor_copy(bc_sb, bc_ps)
                        if g % 2 == 0:
                            nc.vector.tensor_mul(
                                at_sb[g // 2][:DH, :], u_ps[:DH, :], bc_sb
                            )
                        else:
                            at_tmp = rpool.tile([DH, NB], BF16, tag="at_tmp")
                            nc.vector.tensor_mul(at_tmp, u_ps[:DH, :], bc_sb)
                            nc.sync.dma_start(
                                out=at_sb[g // 2][DH : 2 * DH, :], in_=at_tmp
                            )

                    # stage attnT for the cross-core gather
                    c0 = b * L + c * NB
                    for t in range(2):
                        nc.sync.dma_start(
                            out=at_in[t * P : (t + 1) * P, c0 : c0 + NB],
                            in_=at_sb[t],
                        )

            # ---------- gather attnT across cores ----------
            nc.gpsimd.collective_compute(
                "AllGather",
                mybir.AluOpType.bypass,
                replica_groups=groups,
                ins=[at_in.opt()],
                outs=[at_all.opt()],
            )
            # global q-dim chunk j = rows j*128..(j+1)*128 of at_all
            at_r = at_all.rearrange("(k p) l -> p k l", p=P)  # [128, 16, BL]

            # ---------- phase C: disjoint output column slice ----------
            for lb in range(BL // P):
                atg = atgpool.tile([P, KD, P], BF16, tag="atg")
                nc.sync.dma_start(out=atg, in_=at_r[:, :, lb * P : (lb + 1) * P])
                o_ps = psA.tile([P, DQ], F32, tag="acc")
                for k in range(KD):
                    nc.tensor.matmul(
                        o_ps,
                        lhsT=atg[:, k, :],
                        rhs=wo_sb[:, k, :],
                        start=(k == 0),
                        stop=(k == KD - 1),
                    )
                o_sb = opool.tile([P, DQ], BF16, tag="o")
                nc.vector.tensor_copy(o_sb, o_ps)
                nc.sync.dma_start(out=out[lb * P : (lb + 1) * P, :], in_=o_sb)
    nc.compile()
    return nc


def kernel(x, Wq, Wk, Wv, Wo, trace=False):
    x = np.ascontiguousarray(np.asarray(x, dtype=np.float32))
    Wq = np.asarray(Wq, dtype=np.float32).astype(ml_dtypes.bfloat16)
    Wk = np.asarray(Wk, dtype=np.float32).astype(ml_dtypes.bfloat16)
    Wv = np.asarray(Wv, dtype=np.float32).astype(ml_dtypes.bfloat16)
    Wo = np.asarray(Wo, dtype=np.float32).astype(ml_dtypes.bfloat16)

    xb = x.reshape(BL, D).astype(ml_dtypes.bfloat16)  # natural rows; device transposes

    in_maps = []
    for i in range(NCORES):
        qs = slice(i * DQ, (i + 1) * DQ)
        ks = slice(i * DH, (i + 1) * DH)
        in_maps.append(
            {
                "xs": xb[i * NB : (i + 1) * NB],
                "wq": np.ascontiguousarray(Wq[:, qs]),
                "wk": np.ascontiguousarray(Wk[:, ks]),
                "wv": np.ascontiguousarray(Wv[:, ks]),
                "wo": np.ascontiguousarray(Wo[:, qs]),
            }
        )

    if "nc" not in _CACHED:
        _CACHED["nc"] = build_nc()
    nc = _CACHED["nc"]

    res = run_bass_kernel_spmd(nc, in_maps, list(range(NCORES)), trace=trace)
    acc = np.concatenate(
        [np.asarray(r["out"]) for r in res.results], axis=1
    ).astype(np.float32)
    if trace:
        kernel.last_exec_time_ns = res.exec_time_ns
        kernel.last_results = res
    return acc.reshape(B, L, D)


# revision 8
# speedup vs baseline: 9.2553x; 1.0416x over previous
"""GQA kernel for trn2: B=2, L=2048, D=2048, Hq=32, Hkv=8, dh=64.

Sharding: 1 KV head (= 4 contiguous Q heads) per core; Wq/Wk/Wv
column-sharded by head. To minimize host<->device traffic over the
axon/PJRT tunnel (the wall-clock bottleneck), x is uploaded
sequence-sharded (one 512-column slice of xT per core, AllGathered on
device) and the output is produced as disjoint per-core column slices:
the per-core attention outputs (attnT, [256, BL] bf16) are AllGathered
on device and each core contracts the full gathered attnT against its
column shard of Wo, writing out[:, c*256:(c+1)*256] in bf16. The host
just concatenates.

Layout trick: x is transposed on the host (xT: [D, B*L]) so every
on-device matmul has its contraction dim on partitions without any
on-device transposes:
  Q^T[dq, l]  = (Wq_tile).T @ xT        (lhsT=Wq, rhs=xT)
  K^T[dh, l]  = (Wk_tile).T @ xT
  V[l, dh]    = (xT_tile).T @ Wv        (lhsT=xT, rhs=Wv)
  S^T[k, q]   = (K^T_tile).T @ Q^T      (lhsT=K^T, rhs=Q^T)   contract dh=64
  E           = exp(S^T / 8)            (ScalarE, PSUM->SBUF)
  U[0:65, q]  = [V|1].T @ E             (lhsT=V_aug, rhs=E)   contract Lk
                row 64 of U = softmax denominator (ones column trick)
  attnT       = U[:64] * bcast(1/U[64]) (DVE recip + K=1 matmul bcast + mul)
  out[l, mc] += (attnT_all_tile).T @ Wo[:, mc]   (contract full q-dim 2048)
"""

import os
import tempfile

import ml_dtypes
import numpy as np

import jax

# Persistent compilation cache: run_bass_kernel_spmd re-jits per call; a
# disk hit skips the client-side BIR reprocessing (~0.3s/call).
jax.config.update(
    "jax_compilation_cache_dir",
    os.path.join(tempfile.gettempdir(), "jax_comp_cache"),
)
jax.config.update("jax_persistent_cache_min_entry_size_bytes", -1)
jax.config.update("jax_persistent_cache_min_compile_time_secs", 0)

import concourse.bass as bass
import concourse.bacc as bacc
import concourse.mybir as mybir
from concourse.tile import TileContext
from concourse.bass_utils import run_bass_kernel_spmd

B, L, D = 2, 2048, 2048
HQ, HKV, DH = 32, 8, 64
GQ = HQ // HKV            # 4 q heads per core
DQ = GQ * DH              # 256
BL = B * L                # 4096
P = 128
NB = 512                  # free-dim block
KD = D // P               # 16 contraction tiles over D
LT = L // P               # 16 Lk tiles per batch
NBLK = L // NB            # 4 Lq blocks per batch
NCORES = HKV              # 8
SCALE = 1.0 / 8.0         # 1/sqrt(dh)

F32 = mybir.dt.float32
BF16 = mybir.dt.bfloat16
AF = mybir.ActivationFunctionType

_CACHED = {}


def build_nc():
    nc = bacc.Bacc()
    xs = nc.declare_dram_parameter("xs", [NB, D], BF16, isOutput=False)
    wq = nc.declare_dram_parameter("wq", [D, DQ], BF16, isOutput=False)
    wk = nc.declare_dram_parameter("wk", [D, DH], BF16, isOutput=False)
    wv = nc.declare_dram_parameter("wv", [D, DH], BF16, isOutput=False)
    wo = nc.declare_dram_parameter("wo", [D, DQ], BF16, isOutput=False)
    out = nc.declare_dram_parameter("out", [BL, DQ], BF16, isOutput=True)

    groups = [list(range(NCORES))]

    with TileContext(nc) as tc:
        with (
            tc.tile_pool(name="dram", bufs=1, space="DRAM") as dram,
            tc.tile_pool(name="wpool", bufs=1) as wpool,
            tc.tile_pool(name="xpool", bufs=3) as xpool,
            tc.tile_pool(name="qtpool", bufs=3) as qtpool,
            tc.tile_pool(name="ktpool", bufs=2) as ktpool,
            tc.tile_pool(name="vpool", bufs=34) as vpool,
            tc.tile_pool(name="epool", bufs=20) as epool,
            tc.tile_pool(name="atpool", bufs=2) as atpool,
            tc.tile_pool(name="atgpool", bufs=3) as atgpool,
            tc.tile_pool(name="opool", bufs=3) as opool,
            tc.tile_pool(name="bcpool", bufs=2) as bcpool,
            tc.tile_pool(name="rpool", bufs=4) as rpool,
            tc.tile_pool(name="psA", bufs=2, space="PSUM") as psA,
            tc.tile_pool(name="psS", bufs=4, space="PSUM") as psS,
            tc.tile_pool(name="psU", bufs=2, space="PSUM") as psU,
        ):
            # ---- transpose the natural-layout x shard on device (XBAR),
            # then gather the sequence-sharded xT across cores ----
            xin = dram.tile([D, NB], BF16, tag="xin")
            xg = dram.tile([NCORES * D, NB], BF16, tag="xg")
            with tc.tile_pool(name="trpool", bufs=4) as trpool:
                for k in range(KD):
                    tr = trpool.tile([P, NB], BF16, tag="tr", name=f"tr{k}")
                    nc.sync.dma_start_transpose(
                        out=tr, in_=xs[:, k * P : (k + 1) * P]
                    )
                    nc.sync.dma_start(out=xin[k * P : (k + 1) * P, :], in_=tr)
            nc.gpsimd.collective_compute(
                "AllGather",
                mybir.AluOpType.bypass,
                replica_groups=groups,
                ins=[xin.opt()],
                outs=[xg.opt()],
            )
            # shard s of xg = xT[:, s*NB:(s+1)*NB]
            xg_r = xg.rearrange("(s k p) n -> p (s k) n", s=NCORES, p=P)

            # attnT staging (collective in/out), per batch so each gather
            # overlaps the other batch's compute
            at_in = [
                dram.tile([DQ, L], BF16, tag=f"at_in{b}", name=f"at_in{b}")
                for b in range(B)
            ]
            at_all = [
                dram.tile([NCORES * DQ, L], BF16, tag=f"at_all{b}", name=f"at_all{b}")
                for b in range(B)
            ]

            # ---- persistent weights ----
            wq_sb = wpool.tile([P, KD, DQ], BF16, tag="wq")
            nc.sync.dma_start(out=wq_sb, in_=wq.rearrange("(k p) m -> p k m", p=P))
            wk_sb = wpool.tile([P, KD, 2 * DH], BF16, tag="wk")
            nc.sync.dma_start(
                out=wk_sb[:, :, 0:DH], in_=wk.rearrange("(k p) m -> p k m", p=P)
            )
            nc.sync.dma_start(
                out=wk_sb[:, :, DH : 2 * DH],
                in_=wk.rearrange("(k p) m -> p k m", p=P),
            )
            wv_sb = wpool.tile([P, KD, DH], BF16, tag="wv")
            nc.sync.dma_start(out=wv_sb, in_=wv.rearrange("(k p) m -> p k m", p=P))
            wo_sb = wpool.tile([P, KD, DQ], BF16, tag="wo")
            nc.sync.dma_start(out=wo_sb, in_=wo.rearrange("(k p) m -> p k m", p=P))
            ones_sb = wpool.tile([1, DH], BF16, tag="ones")
            nc.vector.memset(ones_sb, 1.0)

            for b in range(B):
                # ---------- phase A: projections for batch b ----------
                qt_sb = [qtpool.tile([P, L], BF16, tag="qt", name=f"qt_sb{t}") for t in range(2)]
                kt_sb = ktpool.tile([P, L], BF16, tag="kt")
                v_sb = [vpool.tile([P, DH + 1], BF16, tag="v", name=f"v_sb{k}") for k in range(LT)]

                for c in range(NBLK):
                    s = b * NBLK + c  # global 512-col block == gather shard
                    xt_all = xpool.tile([P, KD, NB], BF16, tag="xt")
                    nc.sync.dma_start(
                        out=xt_all, in_=xg_r[:, s * KD : (s + 1) * KD, :]
                    )

                    # Q^T (two 128-row dq tiles)
                    for t in range(2):
                        q_ps = psA.tile([P, NB], F32, tag="acc")
                        for k in range(KD):
                            nc.tensor.matmul(
                                q_ps,
                                lhsT=wq_sb[:, k, t * P : (t + 1) * P],
                                rhs=xt_all[:, k, :],
                                start=(k == 0),
                                stop=(k == KD - 1),
                            )
                        nc.vector.tensor_copy(qt_sb[t][:, c * NB : (c + 1) * NB], q_ps)
                    # K^T
                    k_ps = psA.tile([P, NB], F32, tag="acc")
                    for k in range(KD):
                        nc.tensor.matmul(
                            k_ps,
                            lhsT=wk_sb[:, k, :],
                            rhs=xt_all[:, k, :],
                            start=(k == 0),
                            stop=(k == KD - 1),
                        )
                    nc.vector.tensor_copy(kt_sb[:, c * NB : (c + 1) * NB], k_ps)
                    # V (natural, Lk-major) + ones column
                    for j in range(NB // P):
                        lk = c * (NB // P) + j
                        v_ps = psA.tile([P, DH], F32, tag="acc")
                        for k in range(KD):
                            nc.tensor.matmul(
                                v_ps,
                                lhsT=xt_all[:, k, j * P : (j + 1) * P],
                                rhs=wv_sb[:, k, :],
                                start=(k == 0),
                                stop=(k == KD - 1),
                            )
                        nc.vector.tensor_copy(v_sb[lk][:, :DH], v_ps)
                        nc.vector.memset(v_sb[lk][:, DH : DH + 1], 1.0)

                # ---------- phase B per Lq block ----------
                for c in range(NBLK):
                    at_sb = [atpool.tile([P, NB], BF16, tag="at", name=f"at_sb{t}") for t in range(2)]
                    for g in range(GQ):
                        qg = qt_sb[g // 2][
                            (g % 2) * DH : (g % 2) * DH + DH, c * NB : (c + 1) * NB
                        ]
                        # S^T tiles + exp; interleave PV to keep PE/ACT in step
                        e_sb = []
                        u_ps = psU.tile([P, NB], F32, tag="u")

                        h0 = (g % 2) * DH

                        def qk_step(k):
                            sT = psS.tile([P, NB], F32, tag="sT")
                            nc.tensor.matmul(
                                sT,
                                lhsT=kt_sb[h0 : h0 + DH, k * P : (k + 1) * P],
                                rhs=qg,
                                start=True,
                                stop=True,
                            )
                            e = epool.tile([P, NB], BF16, tag="e")
                            nc.scalar.activation(e, sT, AF.Exp, scale=SCALE)
                            e_sb.append(e)

                        def pv_step(k):
                            nc.tensor.matmul(
                                u_ps[: DH + 1, :],
                                lhsT=v_sb[k][:, :],
                                rhs=e_sb[k],
                                start=(k == 0),
                                stop=(k == LT - 1),
                            )

                        for k in range(4):
                            qk_step(k)
                        for k in range(4, LT):
                            qk_step(k)
                            pv_step(k - 4)
                        for k in range(LT - 4, LT):
                            pv_step(k)

                        # normalize: attnT = U[:64] * bcast(1 / U[64])
                        recip = rpool.tile([1, NB], BF16, tag="r")
                        with nc.allow_low_precision(reason="f32r is fp32-width"):
                            nc.vector.reciprocal(recip, u_ps[DH : DH + 1, :])
                        bc_ps = psS.tile([DH, NB], F32, tag="sT")
                        nc.tensor.matmul(
                            bc_ps, lhsT=ones_sb, rhs=recip, start=True, stop=True
                        )
                        bc_sb = bcpool.tile([DH, NB], F32, tag="bc")
                        nc.vector.tensor_copy(bc_sb, bc_ps)
                        if g % 2 == 0:
                            nc.vector.tensor_mul(
                                at_sb[g // 2][:DH, :], u_ps[:DH, :], bc_sb
                            )
                        else:
                            at_tmp = rpool.tile([DH, NB], BF16, tag="at_tmp")
                            nc.vector.tensor_mul(at_tmp, u_ps[:DH, :], bc_sb)
                            nc.sync.dma_start(
                                out=at_sb[g // 2][DH : 2 * DH, :], in_=at_tmp
                            )

                    # stage attnT for the cross-core gather
                    c0 = c * NB
                    for t in range(2):
                        nc.sync.dma_start(
                            out=at_in[b][t * P : (t + 1) * P, c0 : c0 + NB],
                            in_=at_sb[t],
                        )

                # gather this batch's attnT across cores; batch 0's gather
                # overlaps batch 1's phases A+B, batch 1's overlaps phase C0
                nc.gpsimd.collective_compute(
                    "AllGather",
                    mybir.AluOpType.bypass,
                    replica_groups=groups,
                    ins=[at_in[b].opt()],
                    outs=[at_all[b].opt()],
                )

            # ---------- phase C: disjoint output column slice ----------
            for b in range(B):
                # global q-dim chunk j = rows j*128..(j+1)*128 of at_all[b]
                at_r = at_all[b].rearrange("(k p) l -> p k l", p=P)  # [128, 16, L]
                for lb in range(L // P):
                    atg = atgpool.tile([P, KD, P], BF16, tag="atg")
                    nc.sync.dma_start(out=atg, in_=at_r[:, :, lb * P : (lb + 1) * P])
                    o_ps = psA.tile([P, DQ], F32, tag="acc")
                    for k in range(KD):
                        nc.tensor.matmul(
                            o_ps,
                            lhsT=atg[:, k, :],
                            rhs=wo_sb[:, k, :],
                            start=(k == 0),
                            stop=(k == KD - 1),
                        )
                    o_sb = opool.tile([P, DQ], BF16, tag="o")
                    nc.vector.tensor_copy(o_sb, o_ps)
                    row0 = b * L + lb * P
                    nc.sync.dma_start(out=out[row0 : row0 + P, :], in_=o_sb)
    nc.compile()
    return nc


def kernel(x, Wq, Wk, Wv, Wo, trace=False):
    x = np.ascontiguousarray(np.asarray(x, dtype=np.float32))
    Wq = np.asarray(Wq, dtype=np.float32).astype(ml_dtypes.bfloat16)
    Wk = np.asarray(Wk, dtype=np.float32).astype(ml_dtypes.bfloat16)
    Wv = np.asarray(Wv, dtype=np.float32).astype(ml_dtypes.bfloat16)
    Wo = np.asarray(Wo, dtype=np.float32).astype(ml_dtypes.bfloat16)

    xb = x.reshape(BL, D).astype(ml_dtypes.bfloat16)  # natural rows; device transposes

    in_maps = []
    for i in range(NCORES):
        qs = slice(i * DQ, (i + 1) * DQ)
        ks = slice(i * DH, (i + 1) * DH)
        in_maps.append(
            {
                "xs": xb[i * NB : (i + 1) * NB],
                "wq": np.ascontiguousarray(Wq[:, qs]),
                "wk": np.ascontiguousarray(Wk[:, ks]),
                "wv": np.ascontiguousarray(Wv[:, ks]),
                "wo": np.ascontiguousarray(Wo[:, qs]),
            }
        )

    if "nc" not in _CACHED:
        _CACHED["nc"] = build_nc()
    nc = _CACHED["nc"]

    res = run_bass_kernel_spmd(nc, in_maps, list(range(NCORES)), trace=trace)
    acc = np.concatenate(
        [np.asarray(r["out"]) for r in res.results], axis=1
    ).astype(np.float32)
    if trace:
        kernel.last_exec_time_ns = res.exec_time_ns
        kernel.last_results = res
    return acc.reshape(B, L, D)
